# revision 3
# baseline (speedup 1.0000x reference)
"""Depth-gated 3x3 conv (DepthConv) Trainium2 Bass kernel, v2.

Problem: out[b,o,h,w] = sum_{c,kh,kw} x[b,c,h+kh-1,w+kw-1]
                        * exp(-|d[b,h,w] - d[b,h+kh-1,w+kw-1]|)
                        * weight[o,c,kh,kw]  + bias[o]
with B=8, Cin=Cout=64, H=W=128, zero padding.

Sharding: data-parallel over batch, one image per NeuronCore (8 cores).

Key structural facts exploited:
  * The CENTER tap's gate is exp(-|d-d|) = 1, so tap 4 needs no gating at
    all: its raw x window feeds the GEMM directly (no broadcast, no DVE).
  * The remaining 8 taps form a perfect matching compatible with the two
    x staging shifts (+1 elem / +1 row): pairs (0,1),(7,8) on buffer A
    and (2,5),(3,6) on buffer B.

Per-core pipeline (per 1024-output group; 16 groups):
  1. gates g = exp(-|d_center - d_tap|) computed once on 64 partitions
     (8 taps x 8 row-blocks) via DVE sub, ACT abs, ACT exp; relayed to
     partition base 0 by one small DMA per (block, half).
  2. PE "ones-matmul" broadcasts each pair's gates across the channel
     partition dim into PSUM: 4 chunk broadcasts x 2 waves.
  3. Gating multiplies: chunks (0,1),(7,8) via ACT psum->SBUF copy then
     one wide 2x-rate DVE TT; chunk (2,5) DVE TT straight from PSUM;
     chunk (3,6) GpSimd TT straight from PSUM (load-balanced engines).
  4. PE GEMM: 4 pair chunks [128->64] + 1 ungated center chunk [64->64]
     accumulated per 512-wide PSUM tile. Group g's broadcasts are issued
     before group g-1's GEMM so the PE never idles (keeps the 2.4 GHz
     p-state once ramped; a constant warmup ramps it during startup).
  5. ACT adds bias while copying PSUM->SBUF bf16; DMA to DRAM.
"""

import numpy as np

B, CIN, COUT, H, W = 8, 64, 64, 128, 128
HP, WP = H + 2, W + 2            # padded
NPAD = HP * WP                   # 16900
NXCOL = 16904                    # x staging buffer columns (padded + slack)
S = H * W                        # 16384 outputs per image
NB = 8                           # h-blocks
BH = H // NB                     # 16 rows per block
BLK = BH * W                     # 2048 outputs per block
TW = 512                         # psum tile width
GW = 2 * TW                      # group width (2 psum tiles)
DCOL = 4352                      # d64 staging columns
DWIN = (BH - 1) * WP + W         # 2078 valid window elems per block
BANDC = 2368                     # band buffer cols
BANDV = (BH + 1) * WP + W + 2    # 2340 band cols actually loaded

# tap pairing: chunks 0..3 gate tap pairs, chunk 4 is the ungated center.
# (a, a+1) pairs ride staging buffer A (+1 elem), (a, a+3) pairs buffer B
# (+1 row = +130 elems).
PAIRS = [(0, 1), (7, 8), (2, 5), (3, 6)]
CHUNK_BUF = ["A", "A", "B", "B"]
TAPS = [0, 1, 2, 3, 5, 6, 7, 8]          # g row order (center excluded)
RIDX = {t: i for i, t in enumerate(TAPS)}


def _split_multi_waits(nc, mybir):
    """Walrus in this toolchain encodes at most ONE sync wait per
    instruction.  Tile emits multi-wait sync_info; split the extras into
    single-wait NOPs queued just before on the same engine (identical
    semantics: the engine queue blocks on each wait in turn)."""
    cnt = 0
    for f in nc.m.functions:
        for bb in f.blocks:
            newl = []
            for ins in bb.instructions:
                si = ins.sync_info
                if si is not None and si.on_wait and len(si.on_wait) > 1:
                    waits = list(si.on_wait)
                    for w in waits[:-1]:
                        cnt += 1
                        newl.append(
                            mybir.InstNoOp(
                                name=f"waitsplit-{cnt}",
                                ins=[],
                                outs=[],
                                engine=ins.engine,
                                sync_info=mybir.SyncInfo(on_wait=[w], on_update=[]),
                            )
                        )
                    ins.sync_info = mybir.SyncInfo(
                        on_wait=[waits[-1]], on_update=list(si.on_update)
                    )
                newl.append(ins)
            bb.instructions = newl
    return cnt


def build_nc():
    import concourse.bass as bass
    import concourse.mybir as mybir
    from concourse import tile

    f32 = mybir.dt.float32
    bf16 = mybir.dt.bfloat16
    Alu = mybir.AluOpType
    Act = mybir.ActivationFunctionType

    nc = bass.Bass()
    xa_d = nc.declare_dram_parameter("xa", [128, NXCOL], bf16, isOutput=False)
    xb_d = nc.declare_dram_parameter("xb", [128, NXCOL], bf16, isOutput=False)
    d64_d = nc.declare_dram_parameter("d64", [64, DCOL], f32, isOutput=False)
    wt_d = nc.declare_dram_parameter("wt", [640, 64], bf16, isOutput=False)
    em_d = nc.declare_dram_parameter("em", [8, 512], bf16, isOutput=False)
    bias_d = nc.declare_dram_parameter("bias", [128], f32, isOutput=False)
    out_d = nc.declare_dram_parameter("out", [64, S], bf16, isOutput=True)

    with tile.TileContext(nc) as tc:
        with (
            tc.tile_pool(name="consts", bufs=1) as consts,
            tc.tile_pool(name="bands", bufs=2) as bands,
            tc.tile_pool(name="imp", bufs=2) as imp,
            tc.tile_pool(name="gsp", bufs=2) as gsp,
            tc.tile_pool(name="outp", bufs=3) as outp,
            tc.tile_pool(name="pgp", bufs=6, space=bass.MemorySpace.PSUM) as pgp,
            tc.tile_pool(name="pop", bufs=2, space=bass.MemorySpace.PSUM) as pop,
        ):
            # ---- constants ----
            wt_sb = consts.tile([128, 320], bf16, tag="wt")
            nc.sync.dma_start(
                out=wt_sb[:].rearrange("p (j o) -> p j o", o=64),
                in_=wt_d.rearrange("(j p) o -> p j o", p=128),
            )
            em_sb = consts.tile([8, 512], bf16, tag="em")
            nc.sync.dma_start(out=em_sb[:], in_=em_d[:])
            bias_sb = consts.tile([128, 1], f32, tag="bias")
            nc.sync.dma_start(out=bias_sb[:], in_=bias_d.rearrange("(p o) -> p o", o=1))
            d64_sb = consts.tile([64, DCOL], f32, tag="d64")
            nc.sync.dma_start(out=d64_sb[:], in_=d64_d[:])

            # ---- gates: g = exp(-|d_tap - d_center|) on 64 partitions,
            #      computed in two 1024-wide stages so stage-0 gates are
            #      ready early ----
            gdel = consts.tile([64, BLK], f32, tag="gdel")
            gfin = consts.tile([64, BLK], f32, tag="gfin")
            gexp = consts.tile([64, BLK], bf16, tag="gexp")
            win_s = d64_sb[:, 0:BH * WP].rearrange("q (r w) -> q r w", w=WP)[:, :, :W]
            win_c = d64_sb[:, 2176:2176 + BH * WP].rearrange(
                "q (r w) -> q r w", w=WP
            )[:, :, :W]
            gdel_v = gdel[:].rearrange("q (r w) -> q r w", w=W)
            g9t = {}
            for st in range(2):
                cols = slice(GW * st, GW * (st + 1))
                rows = slice(8 * st, 8 * (st + 1))
                nc.vector.tensor_sub(
                    gdel_v[:, rows, :], win_s[:, rows, :], win_c[:, rows, :]
                )
                nc.scalar.activation(gfin[:, cols], gdel[:, cols], Act.Abs)
                nc.scalar.activation(gexp[:, cols], gfin[:, cols], Act.Exp, scale=-1.0)
                for hb in range(NB):
                    g9 = consts.tile([8, GW], bf16, tag=f"g9_{hb}_{st}")
                    nc.sync.dma_start(
                        out=g9[:], in_=gexp[8 * hb:8 * hb + 8, cols]
                    )
                    g9t[(hb, st)] = g9

            # ---- PE clock warmup: dense matmuls gated only on the (tiny,
            #      early) em DMA, so the PE ramps to full clock while the
            #      gate chain runs on the other engines ----
            wu = pgp.tile([128, TW], f32, tag="pg")
            for _ in range(18):
                nc.tensor.matmul(
                    wu[:, :],
                    em_sb[0:8, 0:128],
                    em_sb[0:8, 0:TW],
                    start=True,
                    stop=True,
                    tile_position=(0, 0),
                    skip_group_check=True,
                )

            def load_bands(hb):
                ba = bands.tile([128, BANDC], bf16, tag="bandA")
                nc.sync.dma_start(
                    out=ba[:, :BANDV], in_=xa_d[:, 2080 * hb:2080 * hb + BANDV]
                )
                bb = bands.tile([128, BANDC], bf16, tag="bandB")
                nc.sync.dma_start(
                    out=bb[:, :BANDV], in_=xb_d[:, 2080 * hb:2080 * hb + BANDV]
                )
                return ba, bb

            band_t = {0: load_bands(0)}

            # ---- main loop: 16 groups of 1024 outputs; group g's gate
            #      broadcasts + gating run ahead of group g-1's GEMM ----
            groups = [(hb, q2) for hb in range(NB) for q2 in range(2)]
            prev = None
            for g, (hb, q2) in enumerate(groups):
                if q2 == 0 and hb + 1 < NB:
                    band_t[hb + 1] = load_bands(hb + 1)
                banda, bandb = band_t[hb]
                g9 = g9t[(hb, q2)]

                # 1) gate broadcasts -> PSUM f32 [128,512] per chunk/wave
                pgs = [[None] * 4, [None] * 4]
                for w in range(2):
                    for j in range(4):
                        pg = pgp.tile([128, TW], f32, tag="pg")
                        nc.tensor.matmul(
                            pg[:, :],
                            em_sb[0:8, 128 * j:128 * j + 128],
                            g9[0:8, TW * w:TW * (w + 1)],
                            start=True,
                            stop=True,
                            tile_position=(0, 0),
                            skip_group_check=True,
                        )
                        pgs[w][j] = pg

                # 2) chunks 0,1: ACT copy psum->SBUF bf16 (enables 2x TT)
                gss = {}
                for j in (0, 1):
                    gs = gsp.tile([128, GW], bf16, tag=f"gs{j}")
                    nc.scalar.copy(gs[:, 0:TW], pgs[0][j][:, :])
                    nc.scalar.copy(gs[:, TW:GW], pgs[1][j][:, :])
                    gss[j] = gs

                # 3) gated im2col into bf16 SBUF
                ims = []
                for j in range(4):
                    kh, kw = divmod(PAIRS[j][0], 3)
                    band = banda if CHUNK_BUF[j] == "A" else bandb
                    im = imp.tile([128, GW], bf16, tag=f"im{j}")
                    if j < 2:
                        # all-SBUF wide TT: 2x on DVE; GpSimd takes chunk 1
                        eng = nc.vector if j == 0 else nc.gpsimd
                        off = (8 * q2 + kh) * WP + kw
                        bw = band[0:128, off:off + 1040].rearrange(
                            "p (r w) -> p r w", w=WP
                        )[:, :8, :W]
                        gsv = gss[j][:].rearrange("p (r w) -> p r w", w=W)
                        imv = im[:].rearrange("p (r w) -> p r w", w=W)
                        eng.tensor_tensor(imv, bw, gsv, Alu.mult)
                    else:
                        # PSUM-direct TTs (1x) stay on DVE
                        eng = nc.vector
                        for w in range(2):
                            off = (8 * q2 + 4 * w + kh) * WP + kw
                            bw = band[0:128, off:off + 520].rearrange(
                                "p (r w) -> p r w", w=WP
                            )[:, :4, :W]
                            pgv = pgs[w][j][:, :].rearrange(
                                "p (r w) -> p r w", w=W
                            )
                            imv = im[:, TW * w:TW * (w + 1)].rearrange(
                                "p (r w) -> p r w", w=W
                            )
                            eng.tensor_tensor(imv, bw, pgv, Alu.mult)
                    ims.append(im)

                cur = (ims, banda, hb, q2, g)
                if prev is not None:
                    _emit_main(nc, mybir, wt_sb, bias_sb, out_d, pop, outp, prev)
                prev = cur
            _emit_main(nc, mybir, wt_sb, bias_sb, out_d, pop, outp, prev)
    _split_multi_waits(nc, mybir)
    return nc


def _emit_main(nc, mybir, wt_sb, bias_sb, out_d, pop, outp, state):
    """GEMM + bias + store for one 1024-output group."""
    f32 = mybir.dt.float32
    bf16 = mybir.dt.bfloat16
    Act = mybir.ActivationFunctionType
    ims, banda, hb, q2, g = state
    po = pop.tile([128, TW], f32, tag="po")
    for w in range(2):
        for j in range(5):
            if j < 4:
                lhs = wt_sb[0:128, 64 * j:64 * j + 64]
                mov = ims[j][:, TW * w:TW * (w + 1)]
            else:
                lhs = wt_sb[0:64, 256:320]
                off = (8 * q2 + 4 * w + 1) * WP + 1
                mov = banda[0:64, off:off + 520].rearrange(
                    "p (r w) -> p r w", w=WP
                )[:, :4, :W]
            nc.tensor.matmul(
                po[64 * w:64 * w + 64, :],
                lhs,
                mov,
                start=(j == 0),
                stop=(j == 4),
                tile_position=(0, 64 * w),
                skip_group_check=True,
            )
    ot = outp.tile([128, TW], bf16, tag="ot")
    nc.scalar.activation(ot[:], po[:], Act.Identity, bias=bias_sb[:], scale=1.0)
    nc.sync.dma_start(out=out_d[:, GW * g:GW * g + TW], in_=ot[0:64, :])
    nc.sync.dma_start(out=out_d[:, GW * g + TW:GW * (g + 1)], in_=ot[64:128, :])


# ---------------- host-side input layout prep ----------------

def _pad_flat(img):
    """[C,H,W] -> [C, NPAD] zero-padded flattened."""
    c = img.shape[0]
    p = np.zeros((c, HP, WP), np.float32)
    p[:, 1:1 + H, 1:1 + W] = img
    return p.reshape(c, NPAD)


def prep_x(x_b):
    """x_b [64,H,W] -> xa, xb [128, NXCOL] bf16: lower=padded x, upper
    shifted by +1 / +WP elements."""
    import ml_dtypes

    xp = _pad_flat(np.asarray(x_b, np.float32))
    base = np.zeros((CIN, NXCOL), np.float32)
    base[:, :NPAD] = xp
    upa = np.zeros_like(base)
    upa[:, :NXCOL - 1] = base[:, 1:]
    upb = np.zeros_like(base)
    upb[:, :NXCOL - WP] = base[:, WP:]
    bf = ml_dtypes.bfloat16
    return (
        np.concatenate([base, upa], 0).astype(bf),
        np.concatenate([base, upb], 0).astype(bf),
    )


def prep_d(depth_b):
    """depth_b [H,W] -> d64 [64, DCOL]: per (block, non-center tap) rows
    of shifted + center depth windows."""
    dp = _pad_flat(np.asarray(depth_b, np.float32)[None])[0]
    d64 = np.zeros((64, DCOL), np.float32)
    for hb in range(NB):
        for t in TAPS:
            kh, kw = divmod(t, 3)
            row = 8 * hb + RIDX[t]
            off = 2080 * hb + WP * kh + kw
            d64[row, 0:DWIN] = dp[off:off + DWIN]
            offc = 2080 * hb + WP + 1
            d64[row, 2176:2176 + DWIN] = dp[offc:offc + DWIN]
    return d64


def prep_w(weight):
    """weight [64,64,3,3] -> wt [640,64] chunk-packed (4 tap pairs + the
    center tap), em [8,512] pair-selector."""
    import ml_dtypes

    w2 = np.asarray(weight, np.float32).reshape(COUT, CIN, 9)
    wt = np.zeros((640, 64), ml_dtypes.bfloat16)
    em = np.zeros((8, 512), np.float32)
    for j, (a, b) in enumerate(PAIRS):
        wt[128 * j:128 * j + 64, :] = w2[:, :, a].T
        wt[128 * j + 64:128 * j + 128, :] = w2[:, :, b].T
        em[RIDX[a], 128 * j:128 * j + 64] = 1.0
        em[RIDX[b], 128 * j + 64:128 * j + 128] = 1.0
    wt[512:576, :] = w2[:, :, 4].T
    return wt, em.astype(ml_dtypes.bfloat16)


def make_in_maps(x, depth, weight, bias):
    wt, em = prep_w(weight)
    bias2 = np.ascontiguousarray(np.tile(np.asarray(bias, np.float32), 2))
    in_maps = []
    for b in range(B):
        xa, xb = prep_x(x[b])
        d64 = prep_d(np.asarray(depth)[b, 0])
        in_maps.append(
            {"xa": xa, "xb": xb, "d64": d64, "wt": wt, "em": em, "bias": bias2}
        )
    return in_maps


_NC = None


def run(x, depth, weight, bias, trace=False):
    global _NC
    from concourse.bass_utils import run_bass_kernel_spmd

    if _NC is None:
        _NC = build_nc()
    in_maps = make_in_maps(x, depth, weight, bias)
    res = run_bass_kernel_spmd(_NC, in_maps, list(range(B)), trace=trace)
    out = np.stack(
        [np.asarray(res.results[b]["out"]).reshape(COUT, H, W) for b in range(B)]
    )
    return out.astype(np.float32), res


def kernel(x, depth, weight, bias):
    out, _ = run(x, depth, weight, bias, trace=False)
    return out


# revision 13
# speedup vs baseline: 1.2609x; 1.2609x over previous
"""Depth-gated 3x3 conv (DepthConv) Trainium2 Bass kernel, v2.

Problem: out[b,o,h,w] = sum_{c,kh,kw} x[b,c,h+kh-1,w+kw-1]
                        * exp(-|d[b,h,w] - d[b,h+kh-1,w+kw-1]|)
                        * weight[o,c,kh,kw]  + bias[o]
with B=8, Cin=Cout=64, H=W=128, zero padding.

Sharding: data-parallel over batch, one image per NeuronCore (8 cores).

Key structural facts exploited:
  * The CENTER tap's gate is exp(-|d-d|) = 1, so tap 4 needs no gating at
    all: its raw x window feeds the GEMM directly (no broadcast, no DVE).
  * The remaining 8 taps form a perfect matching compatible with the two
    x staging shifts (+1 elem / +1 row): pairs (0,1),(7,8) on buffer A
    and (2,5),(3,6) on buffer B.

Per-core pipeline (per 1024-output group; 16 groups):
  1. gates g = exp(-|d_center - d_tap|) computed once on 64 partitions
     (8 taps x 8 row-blocks) via DVE sub, ACT abs, ACT exp; relayed to
     partition base 0 by one small DMA per (block, half).
  2. PE "ones-matmul" broadcasts each pair's gates across the channel
     partition dim into PSUM: 4 chunk broadcasts x 2 waves.
  3. Gating multiplies: chunks (0,1),(7,8) via ACT psum->SBUF copy then
     one wide 2x-rate DVE TT; chunk (2,5) DVE TT straight from PSUM;
     chunk (3,6) GpSimd TT straight from PSUM (load-balanced engines).
  4. PE GEMM: 4 pair chunks [128->64] + 1 ungated center chunk [64->64]
     accumulated per 512-wide PSUM tile. Group g's broadcasts are issued
     before group g-1's GEMM so the PE never idles (keeps the 2.4 GHz
     p-state once ramped; a constant warmup ramps it during startup).
  5. ACT adds bias while copying PSUM->SBUF bf16; DMA to DRAM.
"""

import numpy as np

B, CIN, COUT, H, W = 8, 64, 64, 128, 128
HP, WP = H + 2, W + 2            # padded
NPAD = HP * WP                   # 16900
NXCOL = 16904                    # x staging buffer columns (padded + slack)
S = H * W                        # 16384 outputs per image
NB = 8                           # h-blocks
BH = H // NB                     # 16 rows per block
BLK = BH * W                     # 2048 outputs per block
TW = 512                         # psum tile width
GW = 2 * TW                      # group width (2 psum tiles)
DCOL = 4352                      # d64 staging columns
DWIN = (BH - 1) * WP + W         # 2078 valid window elems per block
BANDC = 2368                     # band buffer cols
BANDV = (BH + 1) * WP + W + 2    # 2340 band cols actually loaded

# tap pairing: chunks 0..3 gate tap pairs, chunk 4 is the ungated center.
# (a, a+1) pairs ride staging buffer A (+1 elem), (a, a+3) pairs buffer B
# (+1 row = +130 elems).
PAIRS = [(0, 1), (7, 8), (2, 5), (3, 6)]
CHUNK_BUF = ["A", "A", "B", "B"]
TAPS = [0, 1, 2, 3, 5, 6, 7, 8]          # g row order (center excluded)
RIDX = {t: i for i, t in enumerate(TAPS)}


def _split_multi_waits(nc, mybir):
    """Walrus in this toolchain encodes at most ONE sync wait per
    instruction.  Tile emits multi-wait sync_info; split the extras into
    single-wait NOPs queued just before on the same engine (identical
    semantics: the engine queue blocks on each wait in turn)."""
    cnt = 0
    for f in nc.m.functions:
        for bb in f.blocks:
            newl = []
            for ins in bb.instructions:
                si = ins.sync_info
                if si is not None and si.on_wait and len(si.on_wait) > 1:
                    waits = list(si.on_wait)
                    for w in waits[:-1]:
                        cnt += 1
                        newl.append(
                            mybir.InstNoOp(
                                name=f"waitsplit-{cnt}",
                                ins=[],
                                outs=[],
                                engine=ins.engine,
                                sync_info=mybir.SyncInfo(on_wait=[w], on_update=[]),
                            )
                        )
                    ins.sync_info = mybir.SyncInfo(
                        on_wait=[waits[-1]], on_update=list(si.on_update)
                    )
                newl.append(ins)
            bb.instructions = newl
    return cnt


def build_nc():
    import concourse.bass as bass
    import concourse.mybir as mybir
    from concourse import tile

    f32 = mybir.dt.float32
    bf16 = mybir.dt.bfloat16
    Alu = mybir.AluOpType
    Act = mybir.ActivationFunctionType

    nc = bass.Bass()
    xa_d = nc.declare_dram_parameter("xa", [128, NXCOL], bf16, isOutput=False)
    xb_d = nc.declare_dram_parameter("xb", [128, NXCOL], bf16, isOutput=False)
    d64_d = nc.declare_dram_parameter("d64", [64, DCOL], f32, isOutput=False)
    wt_d = nc.declare_dram_parameter("wt", [640, 64], bf16, isOutput=False)
    em_d = nc.declare_dram_parameter("em", [128, 512], bf16, isOutput=False)
    bias_d = nc.declare_dram_parameter("bias", [128], f32, isOutput=False)
    out_d = nc.declare_dram_parameter("out", [64, S], bf16, isOutput=True)

    with tile.TileContext(nc) as tc:
        with (
            tc.tile_pool(name="consts", bufs=1) as consts,
            tc.tile_pool(name="bands", bufs=2) as bands,
            tc.tile_pool(name="imp", bufs=2) as imp,
            tc.tile_pool(name="gsp", bufs=2) as gsp,
            tc.tile_pool(name="outp", bufs=3) as outp,
            tc.tile_pool(name="pgp", bufs=6, space=bass.MemorySpace.PSUM) as pgp,
            tc.tile_pool(name="pop", bufs=2, space=bass.MemorySpace.PSUM) as pop,
        ):
            # ---- constants (d64 first: it heads the gate-chain critical
            #      path; wt second: the PE warmup hangs off it) ----
            d64_sb = consts.tile([64, DCOL], f32, tag="d64")
            nc.sync.dma_start(out=d64_sb[:], in_=d64_d[:])
            wt_sb = consts.tile([128, 320], bf16, tag="wt")
            nc.sync.dma_start(
                out=wt_sb[:].rearrange("p (j o) -> p j o", o=64),
                in_=wt_d.rearrange("(j p) o -> p j o", p=128),
            )
            em_sb = consts.tile([128, 512], bf16, tag="em")
            nc.sync.dma_start(out=em_sb[:], in_=em_d[:])
            bias_sb = consts.tile([128, 1], f32, tag="bias")
            nc.sync.dma_start(out=bias_sb[:], in_=bias_d.rearrange("(p o) -> p o", o=1))
            # prime the ACT function table off the critical path
            tl_sb = consts.tile([128, 1], f32, tag="tl")
            nc.scalar.activation(tl_sb[:], bias_sb[:], Act.Abs)

            # ---- gates: g = exp(-|d_tap - d_center|) on 64 partitions,
            #      computed in two 1024-wide stages so stage-0 gates are
            #      ready early ----
            gdel = consts.tile([64, BLK], f32, tag="gdel")
            gfin = consts.tile([64, BLK], f32, tag="gfin")
            gexp = consts.tile([64, BLK], bf16, tag="gexp")
            win_s = d64_sb[:, 0:BH * WP].rearrange("q (r w) -> q r w", w=WP)[:, :, :W]
            win_c = d64_sb[:, 2176:2176 + BH * WP].rearrange(
                "q (r w) -> q r w", w=WP
            )[:, :, :W]
            gdel_v = gdel[:].rearrange("q (r w) -> q r w", w=W)
            g9t = {}
            for st in range(2):
                cols = slice(GW * st, GW * (st + 1))
                rows = slice(8 * st, 8 * (st + 1))
                nc.vector.tensor_sub(
                    gdel_v[:, rows, :], win_s[:, rows, :], win_c[:, rows, :]
                )
                nc.scalar.activation(gfin[:, cols], gdel[:, cols], Act.Abs)
                nc.scalar.activation(gexp[:, cols], gfin[:, cols], Act.Exp, scale=-1.0)
                for hb in range(NB):
                    # replicate the 8 gate rows 16x across all 128
                    # partitions (em carries 1/16): the gate-broadcast
                    # matmuls then stream the full PE array height, which
                    # keeps the DVFS high-activity clock state engaged
                    g9 = consts.tile([128, GW], bf16, tag=f"g9_{hb}_{st}")
                    # dst is partition-major [128, GW]; the stride-0 middle
                    # dim of the src feeds each gate row to 16 consecutive
                    # partitions (matching np.repeat in the em prep)
                    nc.sync.dma_start(
                        out=g9[:],
                        in_=gexp[8 * hb:8 * hb + 8, cols]
                        .unsqueeze(1)
                        .broadcast_to([8, 16, GW]),
                    )
                    g9t[(hb, st)] = g9

            # ---- PE clock warmup: full-activity matmuls gated only on the
            #      (early) wt DMA, so the PE ramps to max clock while the
            #      gate chain runs on the other engines ----
            wu = pgp.tile([128, TW], f32, tag="pg")
            for _ in range(24):
                nc.tensor.matmul(
                    wu[:, 0:320],
                    wt_sb[0:128, 0:128],
                    wt_sb[0:128, 0:320],
                    start=True,
                    stop=True,
                    tile_position=(0, 0),
                    skip_group_check=True,
                )

            def load_bands(hb):
                ba = bands.tile([128, BANDC], bf16, tag="bandA")
                nc.sync.dma_start(
                    out=ba[:, :BANDV], in_=xa_d[:, 2080 * hb:2080 * hb + BANDV]
                )
                bb = bands.tile([128, BANDC], bf16, tag="bandB")
                nc.sync.dma_start(
                    out=bb[:, :BANDV], in_=xb_d[:, 2080 * hb:2080 * hb + BANDV]
                )
                return ba, bb

            band_t = {0: load_bands(0)}

            # ---- main loop: 16 groups of 1024 outputs; group g's gate
            #      broadcasts + gating run ahead of group g-1's GEMM ----
            groups = [(hb, q2) for hb in range(NB) for q2 in range(2)]
            prev = None
            for g, (hb, q2) in enumerate(groups):
                if q2 == 0 and hb + 1 < NB:
                    band_t[hb + 1] = load_bands(hb + 1)
                banda, bandb = band_t[hb]
                g9 = g9t[(hb, q2)]

                # 1) gate broadcasts -> PSUM f32 [128,512] per chunk/wave
                pgs = [[None] * 4, [None] * 4]
                for w in range(2):
                    for j in range(4):
                        pg = pgp.tile([128, TW], f32, tag="pg")
                        nc.tensor.matmul(
                            pg[:, :],
                            em_sb[0:128, 128 * j:128 * j + 128],
                            g9[0:128, TW * w:TW * (w + 1)],
                            start=True,
                            stop=True,
                            tile_position=(0, 0),
                            skip_group_check=True,
                        )
                        pgs[w][j] = pg

                # 2) chunks 0,1: ACT copy psum->SBUF bf16 (enables 2x TT)
                gss = {}
                for j in (0, 1):
                    gs = gsp.tile([128, GW], bf16, tag=f"gs{j}")
                    nc.scalar.copy(gs[:, 0:TW], pgs[0][j][:, :])
                    nc.scalar.copy(gs[:, TW:GW], pgs[1][j][:, :])
                    gss[j] = gs

                # 3) gated im2col into bf16 SBUF
                ims = []
                for j in range(4):
                    kh, kw = divmod(PAIRS[j][0], 3)
                    band = banda if CHUNK_BUF[j] == "A" else bandb
                    im = imp.tile([128, GW], bf16, tag=f"im{j}")
                    if j < 2:
                        # all-SBUF wide TT: GpSimd takes chunk 0 (its gs
                        # copies land first, giving the slow engine a head
                        # start; the GEMM consumes chunk 0 last)
                        eng = nc.gpsimd if j == 0 else nc.vector
                        off = (8 * q2 + kh) * WP + kw
                        bw = band[0:128, off:off + 1040].rearrange(
                            "p (r w) -> p r w", w=WP
                        )[:, :8, :W]
                        gsv = gss[j][:].rearrange("p (r w) -> p r w", w=W)
                        imv = im[:].rearrange("p (r w) -> p r w", w=W)
                        eng.tensor_tensor(imv, bw, gsv, Alu.mult)
                    else:
                        # PSUM-direct TTs (1x) stay on DVE
                        eng = nc.vector
                        for w in range(2):
                            off = (8 * q2 + 4 * w + kh) * WP + kw
                            bw = band[0:128, off:off + 520].rearrange(
                                "p (r w) -> p r w", w=WP
                            )[:, :4, :W]
                            pgv = pgs[w][j][:, :].rearrange(
                                "p (r w) -> p r w", w=W
                            )
                            imv = im[:, TW * w:TW * (w + 1)].rearrange(
                                "p (r w) -> p r w", w=W
                            )
                            eng.tensor_tensor(imv, bw, pgv, Alu.mult)
                    ims.append(im)

                cur = (ims, banda, hb, q2, g)
                if prev is not None:
                    _emit_main(nc, mybir, wt_sb, bias_sb, out_d, pop, outp, prev)
                prev = cur
            _emit_main(nc, mybir, wt_sb, bias_sb, out_d, pop, outp, prev)
    _split_multi_waits(nc, mybir)
    return nc


def _emit_main(nc, mybir, wt_sb, bias_sb, out_d, pop, outp, state):
    """GEMM + bias + store for one 1024-output group."""
    f32 = mybir.dt.float32
    bf16 = mybir.dt.bfloat16
    Act = mybir.ActivationFunctionType
    ims, banda, hb, q2, g = state
    po = pop.tile([128, TW], f32, tag="po")
    # GpSimd-built chunk 0 is consumed last; center (j=4) streams all 128
    # band rows against zero-padded weights to stay high-activity
    order = [1, 2, 3, 4, 0]
    for w in range(2):
        for j in order:
            if j < 4:
                lhs = wt_sb[0:128, 64 * j:64 * j + 64]
                mov = ims[j][:, TW * w:TW * (w + 1)]
            else:
                lhs = wt_sb[0:128, 256:320]
                off = (8 * q2 + 4 * w + 1) * WP + 1
                mov = banda[0:128, off:off + 520].rearrange(
                    "p (r w) -> p r w", w=WP
                )[:, :4, :W]
            nc.tensor.matmul(
                po[64 * w:64 * w + 64, :],
                lhs,
                mov,
                start=(j == order[0]),
                stop=(j == order[-1]),
                tile_position=(0, 64 * w),
                skip_group_check=True,
            )
    ot = outp.tile([128, TW], bf16, tag="ot")
    nc.scalar.activation(ot[:], po[:], Act.Identity, bias=bias_sb[:], scale=1.0)
    nc.sync.dma_start(out=out_d[:, GW * g:GW * g + TW], in_=ot[0:64, :])
    nc.sync.dma_start(out=out_d[:, GW * g + TW:GW * (g + 1)], in_=ot[64:128, :])


# ---------------- host-side input layout prep ----------------

def _pad_flat(img):
    """[C,H,W] -> [C, NPAD] zero-padded flattened."""
    c = img.shape[0]
    p = np.zeros((c, HP, WP), np.float32)
    p[:, 1:1 + H, 1:1 + W] = img
    return p.reshape(c, NPAD)


def prep_x(x_b):
    """x_b [64,H,W] -> xa, xb [128, NXCOL] bf16: lower=padded x, upper
    shifted by +1 / +WP elements."""
    import ml_dtypes

    xp = _pad_flat(np.asarray(x_b, np.float32))
    base = np.zeros((CIN, NXCOL), np.float32)
    base[:, :NPAD] = xp
    upa = np.zeros_like(base)
    upa[:, :NXCOL - 1] = base[:, 1:]
    upb = np.zeros_like(base)
    upb[:, :NXCOL - WP] = base[:, WP:]
    bf = ml_dtypes.bfloat16
    return (
        np.concatenate([base, upa], 0).astype(bf),
        np.concatenate([base, upb], 0).astype(bf),
    )


def prep_d(depth_b):
    """depth_b [H,W] -> d64 [64, DCOL]: per (block, non-center tap) rows
    of shifted + center depth windows."""
    dp = _pad_flat(np.asarray(depth_b, np.float32)[None])[0]
    d64 = np.zeros((64, DCOL), np.float32)
    for hb in range(NB):
        for t in TAPS:
            kh, kw = divmod(t, 3)
            row = 8 * hb + RIDX[t]
            off = 2080 * hb + WP * kh + kw
            d64[row, 0:DWIN] = dp[off:off + DWIN]
            offc = 2080 * hb + WP + 1
            d64[row, 2176:2176 + DWIN] = dp[offc:offc + DWIN]
    return d64


def prep_w(weight):
    """weight [64,64,3,3] -> wt [640,64] chunk-packed (4 tap pairs + the
    center tap), em [8,512] pair-selector."""
    import ml_dtypes

    w2 = np.asarray(weight, np.float32).reshape(COUT, CIN, 9)
    wt = np.zeros((640, 64), ml_dtypes.bfloat16)
    em = np.zeros((8, 512), np.float32)
    for j, (a, b) in enumerate(PAIRS):
        wt[128 * j:128 * j + 64, :] = w2[:, :, a].T
        wt[128 * j + 64:128 * j + 128, :] = w2[:, :, b].T
        em[RIDX[a], 128 * j:128 * j + 64] = 1.0
        em[RIDX[b], 128 * j + 64:128 * j + 128] = 1.0
    wt[512:576, :] = w2[:, :, 4].T
    # gate rows are staged 16x-replicated (tap-major interleave) across the
    # 128 moving partitions; 1/16 makes the broadcast an exact average
    em_rep = np.repeat(em, 16, axis=0) / 16.0
    return wt, em_rep.astype(ml_dtypes.bfloat16)


def make_in_maps(x, depth, weight, bias):
    wt, em = prep_w(weight)
    bias2 = np.ascontiguousarray(np.tile(np.asarray(bias, np.float32), 2))
    in_maps = []
    for b in range(B):
        xa, xb = prep_x(x[b])
        d64 = prep_d(np.asarray(depth)[b, 0])
        in_maps.append(
            {"xa": xa, "xb": xb, "d64": d64, "wt": wt, "em": em, "bias": bias2}
        )
    return in_maps


_NC = None


def run(x, depth, weight, bias, trace=False):
    global _NC
    from concourse.bass_utils import run_bass_kernel_spmd

    if _NC is None:
        _NC = build_nc()
    in_maps = make_in_maps(x, depth, weight, bias)
    res = run_bass_kernel_spmd(_NC, in_maps, list(range(B)), trace=trace)
    out = np.stack(
        [np.asarray(res.results[b]["out"]).reshape(COUT, H, W) for b in range(B)]
    )
    return out.astype(np.float32), res


def kernel(x, depth, weight, bias):
    out, _ = run(x, depth, weight, bias, trace=False)
    return out
